# revision 32
# baseline (speedup 1.0000x reference)
"""Attention with host-folded QK^T kernel + pair-wise V dedup AllGather.

v12: the K projection never runs on device. scores = q·k^T with
q = x W_q, k = x W_k factors as x (W_q W_k^T) x^T, so the host
precomputes M = 64·W_q W_k^T (fp32 matmul, then fp16 — the 64×
scale keeps M's ~1e-5-magnitude entries out of fp16 subnormals;
the exp activation scale absorbs the 1/64). Each core computes
Q' = x_q M for its query half (same cost as the old Q projection)
and scores come from Q'·x_k^T against the xh slabs directly —
the entire 32µs redundant full-K projection is gone.

Keys are RANK-RELATIVE: the host builds each core's xh as
[own 1024 tokens | peer 1024 tokens], which (a) makes xq redundant
(A1/A2 read the first two xh slabs), and (b) lets V' for the own
half live entirely on-core (vpA) — only the PEER half of V' comes
back from the 2-rank AllGather. The gather output is rank-ordered,
so the peer slot index depends on the rank; two complementary
cond= predicated DMAs (skipped DMAs still increment their
semaphore) funnel the right slot into vpB with fully static APs.
Attention is an order-free reduction over keys, so rank-relative
key order changes nothing downstream.

Schedule notes (v13):
- ALL input DMAs ride ONE hardware queue (sync) in strict need
  order: the 16 DMA engines are shared across queues, so a
  second concurrent queue steals ~half the bandwidth from the
  critical stream (measured). Chunked wv0/slab0/slab1 keep the
  per-DMA completion semaphores fine-grained, so the first ps1
  matmul waits on ~690KB, not 1.4MB, and ps1-kc4 waits on half
  of slab1, not all of it.
- 10 HAM-warmup matmuls (memset on vector) bridge the ~5.5us
  DMA wait and un-throttle the PE clock gate (K=4/8 1.2GHz ->
  K=8/8 2.4GHz after ~3.4us of sustained activity), so the first
  REAL matmul already runs at 2.4GHz (one K=8 HAM event spans
  the whole kernel in the trace).
- the scalar DMA queue is pre-warmed with a tiny transfer; its
  only other use is the final output chunks, which otherwise pay
  the ~1.6us first-doorbell queue-start latency on the tail.
- ps_sc is allocated OUTSIDE the psa scope and the scores loop
  runs INSIDE it, so no pool-close fence sits between A2 and
  scores (measured 1.05us of PE idle otherwise); the psa close
  lands at the scores->out boundary instead.
- PSUM budget: wps(1) + psa(5) + ps_sc(2) = 8 banks in phase A;
  out phase: ps_den(3x1) + ps_av(3x1) reuse psa's banks.
- out phase: the denominator run (vp cols 512:770) accumulates
  into its OWN 1-bank psum tile (ps_den). With a shared tile the
  0:512 run was WAR-serialized behind the vector normalize that
  reads the denominator half (~0.7us per exposed boundary; Tile
  tracks hazards at tile granularity).
- vpA and vpB are separate tiles: a single tile written by two
  different DMA queues loses one of the matmul waits (HW 1-wait
  limit) — observed as a cold-run race.
- last q-chunk: cols 512:768 ship right after the denominator
  normalize; the 0:512 run is split 256/192/64 with interleaved
  normalize+DMA on alternating warm queues, so only a 64-col DMA
  and one small normalize sit after the final matmul (~1.9us
  tail vs ~3.8us for a monolithic last chunk).
- the output tensor is fp16 (host upcasts to fp32): halves the
  output DMA bytes on the tail and doubles the DVE normalize
  rate; adds ~1e-4 quantization (rel err 8.0e-4 -> 9.1e-4).
- fp8 DoubleRow was evaluated and rejected: e4m3 on any of the
  big matmuls gives 3.3e-2..5.7e-2 max-rel error (near-one-hot
  softmax rows pass single-element quantization error straight
  through) vs the 2e-2 gate.
"""

import numpy as np

import concourse.bass as bass
import concourse.mybir as mybir
import concourse.tile as tile
from concourse import bacc
from concourse.bass_utils import run_bass_kernel_spmd

N_CORES = 8
B, N, D, OUT = 4, 2048, 768, 768
NQ = N // 2
P = 128
DC = D // P
KC = N // P
HKC = KC // 2  # k-chunks per half
F32 = mybir.dt.float32
FP16 = mybir.dt.float16
PAIRS = [[0, 1], [2, 3], [4, 5], [6, 7]]

M_SCALE = 64.0  # host folds this into M; exp scale divides it back out


def build_attention_nc():
    nc = bacc.Bacc("TRN2", target_bir_lowering=False, debug=False)
    # Inputs host-pre-arranged in SBUF slab layout [p, dc, n]; xh is
    # rank-relative: slabs 0-1 = own 1024 tokens, 2-3 = peer tokens.
    xh = nc.dram_tensor("xh", [4, P, DC * 512], FP16, kind="ExternalInput")
    mw = nc.dram_tensor("mw", [P, DC * D], FP16, kind="ExternalInput")
    wvi = nc.dram_tensor("wvi", [P, 2 * DC * 384], FP16, kind="ExternalInput")
    # fp16 output: halves the output DMA bytes (the final chunk sits on
    # the critical tail) and doubles the DVE normalize rate; the ~5e-4
    # fp16 quantization is negligible vs the 2e-2 gate (host upcasts).
    out = nc.dram_tensor("out", [NQ, OUT], FP16, kind="ExternalOutput")

    with tile.TileContext(nc) as tc:
        with (
            tc.tile_pool(name="persist", bufs=1) as persist,
            tc.tile_pool(name="slabs", bufs=4) as slabs,
            tc.tile_pool(name="wpool", bufs=1) as wpool,
            tc.tile_pool(name="expp", bufs=34) as expp,
            tc.tile_pool(name="obp", bufs=3) as obp,
            tc.tile_pool(name="smallp", bufs=4) as smallp,
            tc.tile_pool(name="ps_sc", bufs=2, space="PSUM") as ps_sc,
            tc.tile_pool(name="dpool", bufs=1, space="DRAM") as dpool,
        ):
            # Q'^T[d,q], one tile per 512-query half so the scores phase
            # never waits on the other half's psum drain
            qpt = [
                persist.tile([P, DC, 512], FP16, name=f"qpt{s}")
                for s in range(2)
            ]
            # V' in rank-relative key order: vpA = own half (local only),
            # vpB = peer half (from the gather)
            vpA = persist.tile([P, HKC, OUT + 2], FP16, name="vpA")
            vpB = persist.tile([P, HKC, OUT + 2], FP16, name="vpB")

            vpb_in = dpool.tile([P, HKC, OUT + 2], FP16)
            vpb_out = dpool.tile([2, P, HKC, OUT + 2], FP16)

            wv_sb = wpool.tile([P, 2, DC, 384], FP16)
            mw_sb = wpool.tile([P, DC, D], FP16)

            # HAM warmup; memset on vector (earliest-idle engine) so the
            # ramp matmuls can start right after the preamble barrier.
            warm = wpool.tile([P, 512], FP16, name="warm")
            nc.vector.memset(warm, 1.0)

            # Pre-warm the scalar DMA queue with a tiny transfer: its
            # only other use is the final output chunk, which otherwise
            # pays the ~1.6us first-doorbell queue-start latency right
            # on the critical tail.
            qwarm = wpool.tile([P, 8], FP16, name="qwarm")
            nc.scalar.dma_start(out=qwarm, in_=mw[:, 0:8])

            ones_sc = persist.tile([P, 1], F32, name="ones_sc")
            nc.vector.memset(ones_sc, 1.0)
            zero_sc = persist.tile([P, 1], F32, name="zero_sc")
            nc.vector.memset(zero_sc, 0.0)

            ets = {}
            with tc.tile_pool(name="psa", bufs=5, space="PSUM") as psa:
                wps = psa.tile([P, 512], F32, name="wps", bufs=1)
                for i in range(10):
                    nc.tensor.matmul(
                        wps, warm[:, 0:P], warm, start=(i == 0), stop=(i == 9)
                    )

                # All input DMAs ride ONE queue (sync) in strict NEED
                # order: the 16 DMA engines are shared across queues, so
                # a second concurrent queue would steal ~half the
                # bandwidth from the critical stream (measured: A1
                # starved when mw/slab2 ran on the scalar queue early).
                # Chunking wv0/slab0 lets the first ps1 matmul wait on
                # only ~540KB instead of ~1.4MB.
                kslab_tiles = [
                    slabs.tile([P, 4, DC, P], FP16, tag="slab", name=f"kslab{s}")
                    for s in range(4)
                ]
                nc.sync.dma_start(
                    out=wv_sb[:, 0, 0:4], in_=wvi[:, 0 : 4 * 384]
                )
                nc.sync.dma_start(
                    out=kslab_tiles[0][:, 0:2], in_=xh[0][:, 0 : 2 * DC * P]
                )
                nc.sync.dma_start(
                    out=wv_sb[:, 0, 4:6], in_=wvi[:, 4 * 384 : DC * 384]
                )
                nc.sync.dma_start(
                    out=kslab_tiles[0][:, 2:4], in_=xh[0][:, 2 * DC * P :]
                )
                nc.sync.dma_start(
                    out=kslab_tiles[1][:, 0:2], in_=xh[1][:, 0 : 2 * DC * P]
                )
                nc.sync.dma_start(
                    out=kslab_tiles[1][:, 2:4], in_=xh[1][:, 2 * DC * P :]
                )
                nc.sync.dma_start(
                    out=wv_sb[:, 1], in_=wvi[:, DC * 384 :]
                )
                nc.sync.dma_start(out=mw_sb, in_=mw[:, :])
                nc.sync.dma_start(out=kslab_tiles[3], in_=xh[3])
                nc.sync.dma_start(out=kslab_tiles[2], in_=xh[2])

                # ---- A1: V' own half (earliest -> feeds the gather) ----
                # kc0/kc1 run their wv0a-covered dc 0:4 halves FIRST
                # (two open psum groups on different banks): kc1's dc0-3
                # fills the ~1us wait for the wv0b chunk (dc 4-5) that a
                # straight kc0 dc-loop exposes right at kernel start.
                ps_first = [
                    psa.tile([P, 512], F32, tag="psa", name=f"ps1f{k}")
                    for k in range(2)
                ]
                for k in range(2):
                    for dc in range(4):
                        nc.tensor.matmul(
                            ps_first[k][:, 0:384],
                            kslab_tiles[0][:, k, dc, :],
                            wv_sb[:, 0, dc, :],
                            start=(dc == 0),
                            stop=False,
                        )
                for k in range(2):
                    for dc in range(4, DC):
                        nc.tensor.matmul(
                            ps_first[k][:, 0:384],
                            kslab_tiles[0][:, k, dc, :],
                            wv_sb[:, 0, dc, :],
                            start=False,
                            stop=(dc == DC - 1),
                        )
                    nc.vector.tensor_copy(
                        vpA[:, k, 0:384], ps_first[k][:, 0:384]
                    )
                for kc in range(2, HKC):
                    slab = kslab_tiles[kc // 4]
                    ps1 = psa.tile([P, 512], F32, tag="psa")
                    for dc in range(DC):
                        nc.tensor.matmul(
                            ps1[:, 0:384],
                            slab[:, kc % 4, dc, :],
                            wv_sb[:, 0, dc, :],
                            start=(dc == 0),
                            stop=(dc == DC - 1),
                        )
                    nc.vector.tensor_copy(vpA[:, kc, 0:384], ps1[:, 0:384])
                for kc in range(HKC):
                    slab = kslab_tiles[kc // 4]
                    ps2 = psa.tile([P, 512], F32, tag="psa")
                    for dc in range(DC):
                        nc.tensor.matmul(
                            ps2[:, 0:384],
                            slab[:, kc % 4, dc, :],
                            wv_sb[:, 1, dc, :],
                            start=(dc == 0),
                            stop=(dc == DC - 1),
                        )
                    nc.vector.tensor_copy(vpA[:, kc, 384:OUT], ps2[:, 0:384])
                    nc.vector.tensor_copy(vpA[:, kc, OUT : OUT + 1], ones_sc)
                    nc.vector.tensor_copy(
                        vpA[:, kc, OUT + 1 : OUT + 2], zero_sc
                    )
                    nc.gpsimd.dma_start(
                        out=vpb_in[:, kc, :], in_=vpA[:, kc, :]
                    )
                nc.gpsimd.collective_compute(
                    "AllGather",
                    mybir.AluOpType.bypass,
                    replica_groups=PAIRS,
                    ins=[vpb_in.opt()],
                    outs=[vpb_out.opt()],
                )
                # Peer-half readback: the gather output is rank-ordered,
                # so rank r's peer sits in slot 1-r. Two complementary
                # predicated DMAs keep the APs static; the skipped DMA
                # still increments the semaphore, so downstream waits
                # count identically on both ranks. Both on the sync
                # queue (single-queue writers keep the matmul wait).
                me = nc.sync.partition_id() % 2
                nc.sync.dma_start(out=vpB[:], in_=vpb_out[0], cond=me)
                nc.sync.dma_start(
                    out=vpB[:], in_=vpb_out[1], cond=(me + 1) % 2
                )

                # ---- A2: Q'^T = (x_q M)^T own half ----
                for s in range(2):
                    slab = kslab_tiles[s]
                    for oc in range(DC):
                        ps = psa.tile([P, 512], F32, tag="psa")
                        for dc in range(DC):
                            nc.tensor.matmul(
                                ps,
                                mw_sb[:, dc, oc * P : (oc + 1) * P],
                                slab[:, :, dc, :],
                                start=(dc == 0),
                                stop=(dc == DC - 1),
                            )
                        nc.vector.tensor_copy(qpt[s][:, oc, :], ps)

                # ---- scoresT: contracts over d, stationary = xh slab
                # chunks (rank-relative key order), moving = Q'^T. Runs
                # inside the psa scope (ps_sc has its own banks) so no
                # pool-close fence sits between A2 and scores.
                for bi in range(2):
                    for kc in range(KC):
                        kslab = kslab_tiles[kc // 4]
                        st = ps_sc.tile([P, 512], F32, tag="sc")
                        for dc in range(DC):
                            nc.tensor.matmul(
                                st,
                                kslab[:, kc % 4, dc, :],
                                qpt[bi][:, dc, :],
                                start=(dc == 0),
                                stop=(dc == DC - 1),
                            )
                        et = expp.tile(
                            [P, 512], FP16, tag="exp", name=f"et{bi}_{kc}"
                        )
                        nc.scalar.activation(
                            et,
                            st,
                            mybir.ActivationFunctionType.Exp,
                            scale=0.125 / M_SCALE,
                        )
                        ets[(bi, kc)] = et

            # ---- out phase: psa's banks freed above feed ps_out; the
            # pool-close fence overlaps the V-gather wait.
            with (
                tc.tile_pool(name="ps_den", bufs=3, space="PSUM") as ps_den,
                tc.tile_pool(name="ps_av", bufs=3, space="PSUM") as ps_av,
            ):
                # 8 q-chunks of 128. Denominator run (cols 512:770) goes
                # FIRST into its OWN 1-bank psum tile so the recip and
                # 512:768 normalize never WAR-block the 0:512 run (a
                # shared tile serialized run2 behind the normalize —
                # measured ~0.7us per exposed boundary); kc 0-7 read vpA
                # (local), kc 8-15 read vpB (gathered peer half).
                def vsrc(kc):
                    return vpA if kc < HKC else vpB

                for j in range(NQ // P):
                    bi, jj = j // 4, j % 4
                    opd = ps_den.tile([P, 258], F32, tag="den", name=f"den{j}")
                    for kc in range(KC):
                        nc.tensor.matmul(
                            opd,
                            ets[(bi, kc)][:, jj * P : (jj + 1) * P],
                            vsrc(kc)[:, kc % HKC, 512 : OUT + 2],
                            start=(kc == 0),
                            stop=(kc == KC - 1),
                        )
                    recip = smallp.tile([P, 1], F32, tag="recip")
                    nc.vector.reciprocal(recip, opd[:, 256:257])
                    ob = obp.tile([P, OUT], FP16, tag="ob")
                    nc.vector.tensor_scalar_mul(
                        ob[:, 512:OUT], opd[:, 0:256], recip
                    )
                    if j == NQ // P - 1:
                        # last chunk: ship cols 512:768 now (scalar
                        # queue), run the 0:512 accumulation as two
                        # 256-col runs in separate psum tiles so the
                        # first half's normalize + DMA overlap the
                        # second half's matmuls.
                        nc.scalar.dma_start(
                            out=out[j * P : (j + 1) * P, 512:OUT],
                            in_=ob[:, 512:OUT],
                        )
                        for lo, hi, eng in (
                            (0, 256, nc.sync),
                            (256, 448, nc.scalar),
                            (448, 512, nc.sync),
                        ):
                            opa = ps_av.tile(
                                [P, hi - lo], F32, tag="av", name=f"av{j}_{lo}"
                            )
                            for kc in range(KC):
                                nc.tensor.matmul(
                                    opa,
                                    ets[(bi, kc)][:, jj * P : (jj + 1) * P],
                                    vsrc(kc)[:, kc % HKC, lo:hi],
                                    start=(kc == 0),
                                    stop=(kc == KC - 1),
                                )
                            nc.vector.tensor_scalar_mul(
                                ob[:, lo:hi], opa, recip
                            )
                            eng.dma_start(
                                out=out[j * P : (j + 1) * P, lo:hi],
                                in_=ob[:, lo:hi],
                            )
                    else:
                        opa = ps_av.tile([P, 512], F32, tag="av", name=f"av{j}")
                        for kc in range(KC):
                            nc.tensor.matmul(
                                opa,
                                ets[(bi, kc)][:, jj * P : (jj + 1) * P],
                                vsrc(kc)[:, kc % HKC, 0:512],
                                start=(kc == 0),
                                stop=(kc == KC - 1),
                            )
                        nc.vector.tensor_scalar_mul(
                            ob[:, 0:512], opa, recip
                        )
                        nc.sync.dma_start(
                            out=out[j * P : (j + 1) * P, :], in_=ob
                        )
    nc.finalize()
    return nc


_NC_CACHE = None


def _get_nc():
    global _NC_CACHE
    if _NC_CACHE is None:
        _NC_CACHE = build_attention_nc()
    return _NC_CACHE


def _xh_layout(a2d):
    """[D, 2048] -> [4, P, 4*DC*128], quarter-major slabs: the kc-th
    128-token quarter of a slab is a contiguous DMA prefix."""
    t = a2d.reshape(DC, P, 4, 4, P)  # dc p s q t
    t = t.transpose(2, 1, 3, 0, 4)  # s p q dc t
    return np.ascontiguousarray(t.reshape(4, P, 4 * DC * P))


def _wv_layout(a2d):
    """[D, 768] -> [P, 2*DC*384], column-half-major."""
    t = a2d.reshape(DC, P, 2, 384)  # dc p h c
    t = t.transpose(1, 2, 0, 3)  # p h dc c
    return np.ascontiguousarray(t.reshape(P, 2 * DC * 384))


def _mw_layout(a2d):
    """[D, D] -> [P, DC*D], dc-major."""
    t = a2d.reshape(DC, P, D).transpose(1, 0, 2)
    return np.ascontiguousarray(t.reshape(P, DC * D))


def make_in_maps(x, kernel):
    x = np.asarray(x, dtype=np.float32)
    w = np.asarray(kernel, dtype=np.float32)
    mw16 = (M_SCALE * (w[0] @ w[1].T)).astype(np.float16)
    mw = _mw_layout(mw16)
    wv = _wv_layout(w[2].astype(np.float16))
    in_maps = []
    for core in range(N_CORES):
        b, half = core // 2, core % 2
        xt16 = x[b].T.astype(np.float16)
        # rank-relative key order: own 1024 tokens first, then peer's
        own = xt16[:, half * NQ : (half + 1) * NQ]
        peer = xt16[:, (1 - half) * NQ : (2 - half) * NQ]
        xh = _xh_layout(np.concatenate([own, peer], axis=1))
        in_maps.append({"xh": xh, "mw": mw, "wvi": wv})
    return in_maps


def assemble_output(results):
    out = np.empty((B, N, OUT), dtype=np.float32)
    for core in range(N_CORES):
        b, half = core // 2, core % 2
        out[b, half * NQ : (half + 1) * NQ, :] = results[core]["out"]
    return out


def run_on_hw(x, kernel, trace=False):
    nc = _get_nc()
    res = run_bass_kernel_spmd(
        nc, make_in_maps(x, kernel), list(range(N_CORES)), trace=trace
    )
    return assemble_output(res.results), res


def kernel(x, kernel):
    out, _ = run_on_hw(x, kernel, trace=False)
    return out



# revision 34
# speedup vs baseline: 1.0989x; 1.0989x over previous
"""Attention with host-folded QK^T kernel + pair-wise V dedup AllGather.

v12: the K projection never runs on device. scores = q·k^T with
q = x W_q, k = x W_k factors as x (W_q W_k^T) x^T, so the host
precomputes M = 64·W_q W_k^T (fp32 matmul, then fp16 — the 64×
scale keeps M's ~1e-5-magnitude entries out of fp16 subnormals;
the exp activation scale absorbs the 1/64). Each core computes
Q' = x_q M for its query half (same cost as the old Q projection)
and scores come from Q'·x_k^T against the xh slabs directly —
the entire 32µs redundant full-K projection is gone.

Keys are RANK-RELATIVE: the host builds each core's xh as
[own 1024 tokens | peer 1024 tokens], which (a) makes xq redundant
(A1/A2 read the first two xh slabs), and (b) lets V' for the own
half live entirely on-core (vpA) — only the PEER half of V' comes
back from the 2-rank AllGather. The gather output is rank-ordered,
so the peer slot index depends on the rank; two complementary
cond= predicated DMAs (skipped DMAs still increment their
semaphore) funnel the right slot into vpB with fully static APs.
Attention is an order-free reduction over keys, so rank-relative
key order changes nothing downstream.

Schedule notes (v13):
- ALL input DMAs ride ONE hardware queue (sync) in strict need
  order: the 16 DMA engines are shared across queues, so a
  second concurrent queue steals ~half the bandwidth from the
  critical stream (measured). Chunked wv0/slab0/slab1 keep the
  per-DMA completion semaphores fine-grained, so the first ps1
  matmul waits on ~690KB, not 1.4MB, and ps1-kc4 waits on half
  of slab1, not all of it.
- 10 HAM-warmup matmuls (memset on vector) bridge the ~5.5us
  DMA wait and un-throttle the PE clock gate (K=4/8 1.2GHz ->
  K=8/8 2.4GHz after ~3.4us of sustained activity), so the first
  REAL matmul already runs at 2.4GHz (one K=8 HAM event spans
  the whole kernel in the trace).
- A1 kc0/kc1 run their wv0a-covered dc halves first as two open
  psum groups: kc1's dc0-3 fills the ~1.1us wait for the wv0b
  chunk. With this the whole work span measures within ~0.3us of
  the 270592-column streaming ideal (zero stalls >160ns).
- the scalar DMA queue is pre-warmed with a tiny transfer; its
  only other use is the final output chunks, which otherwise pay
  the ~1.6us first-doorbell queue-start latency on the tail.
- ps_sc is allocated OUTSIDE the psa scope and the scores loop
  runs INSIDE it, so no pool-close fence sits between A2 and
  scores (measured 1.05us of PE idle otherwise); the psa close
  lands at the scores->out boundary instead.
- PSUM budget: wps(1) + psa(5) + ps_sc(2) = 8 banks in phase A;
  out phase: ps_den(3x1) + ps_av(3x1) reuse psa's banks.
- out phase: the denominator run (vp cols 512:770) accumulates
  into its OWN 1-bank psum tile (ps_den). With a shared tile the
  0:512 run was WAR-serialized behind the vector normalize that
  reads the denominator half (~0.7us per exposed boundary; Tile
  tracks hazards at tile granularity).
- vpA and vpB are separate tiles: a single tile written by two
  different DMA queues loses one of the matmul waits (HW 1-wait
  limit) — observed as a cold-run race.
- last q-chunk: cols 512:768 ship right after the denominator
  normalize; the 0:512 run is split 256/192/64 with interleaved
  normalize+DMA on alternating warm queues, so only a 64-col DMA
  and one small normalize sit after the final matmul (~1.9us
  tail vs ~3.8us for a monolithic last chunk).
- the output tensor is fp16 (host upcasts to fp32): halves the
  output DMA bytes on the tail and doubles the DVE normalize
  rate; adds ~1e-4 quantization (rel err 8.0e-4 -> 9.1e-4).
- fp8 DoubleRow was evaluated and rejected: e4m3 on any of the
  big matmuls gives 3.3e-2..5.7e-2 max-rel error (near-one-hot
  softmax rows pass single-element quantization error straight
  through) vs the 2e-2 gate.
"""

import numpy as np

import concourse.bass as bass
import concourse.mybir as mybir
import concourse.tile as tile
from concourse import bacc
from concourse.bass_utils import run_bass_kernel_spmd

N_CORES = 8
B, N, D, OUT = 4, 2048, 768, 768
NQ = N // 2
P = 128
DC = D // P
KC = N // P
HKC = KC // 2  # k-chunks per half
F32 = mybir.dt.float32
FP16 = mybir.dt.float16
PAIRS = [[0, 1], [2, 3], [4, 5], [6, 7]]

M_SCALE = 64.0  # host folds this into M; exp scale divides it back out


def build_attention_nc():
    nc = bacc.Bacc("TRN2", target_bir_lowering=False, debug=False)
    # Inputs host-pre-arranged in SBUF slab layout [p, dc, n]; xh is
    # rank-relative: slabs 0-1 = own 1024 tokens, 2-3 = peer tokens.
    xh = nc.dram_tensor("xh", [4, P, DC * 512], FP16, kind="ExternalInput")
    mw = nc.dram_tensor("mw", [P, DC * D], FP16, kind="ExternalInput")
    wvi = nc.dram_tensor("wvi", [P, 2 * DC * 384], FP16, kind="ExternalInput")
    # fp16 output: halves the output DMA bytes (the final chunk sits on
    # the critical tail) and doubles the DVE normalize rate; the ~5e-4
    # fp16 quantization is negligible vs the 2e-2 gate (host upcasts).
    out = nc.dram_tensor("out", [NQ, OUT], FP16, kind="ExternalOutput")

    with tile.TileContext(nc) as tc:
        with (
            tc.tile_pool(name="persist", bufs=1) as persist,
            tc.tile_pool(name="slabs", bufs=4) as slabs,
            tc.tile_pool(name="wpool", bufs=1) as wpool,
            tc.tile_pool(name="expp", bufs=34) as expp,
            tc.tile_pool(name="obp", bufs=3) as obp,
            tc.tile_pool(name="smallp", bufs=4) as smallp,
            tc.tile_pool(name="ps_sc", bufs=2, space="PSUM") as ps_sc,
            tc.tile_pool(name="dpool", bufs=1, space="DRAM") as dpool,
        ):
            # Q'^T[d,q], one tile per 512-query half so the scores phase
            # never waits on the other half's psum drain
            qpt = [
                persist.tile([P, DC, 512], FP16, name=f"qpt{s}")
                for s in range(2)
            ]
            # V' in rank-relative key order: vpA = own half (local only),
            # vpB = peer half (from the gather)
            vpA = persist.tile([P, HKC, OUT + 2], FP16, name="vpA")
            vpB = persist.tile([P, HKC, OUT + 2], FP16, name="vpB")

            vpb_in = dpool.tile([P, HKC, OUT + 2], FP16)
            vpb_out = dpool.tile([2, P, HKC, OUT + 2], FP16)

            wv_sb = wpool.tile([P, 2, DC, 384], FP16)
            mw_sb = wpool.tile([P, DC, D], FP16)

            # HAM warmup; memset on vector (earliest-idle engine) so the
            # ramp matmuls can start right after the preamble barrier.
            warm = wpool.tile([P, 512], FP16, name="warm")
            nc.vector.memset(warm, 1.0)

            # Pre-warm the scalar DMA queue with a tiny transfer: its
            # only other use is the final output chunk, which otherwise
            # pays the ~1.6us first-doorbell queue-start latency right
            # on the critical tail.
            qwarm = wpool.tile([P, 8], FP16, name="qwarm")
            nc.scalar.dma_start(out=qwarm, in_=mw[:, 0:8])

            ones_sc = persist.tile([P, 1], F32, name="ones_sc")
            nc.vector.memset(ones_sc, 1.0)
            zero_sc = persist.tile([P, 1], F32, name="zero_sc")
            nc.vector.memset(zero_sc, 0.0)

            ets = {}
            with tc.tile_pool(name="psa", bufs=5, space="PSUM") as psa:
                wps = psa.tile([P, 512], F32, name="wps", bufs=1)
                for i in range(10):
                    nc.tensor.matmul(
                        wps, warm[:, 0:P], warm, start=(i == 0), stop=(i == 9)
                    )

                # All input DMAs ride ONE queue (sync) in strict NEED
                # order: the 16 DMA engines are shared across queues, so
                # a second concurrent queue would steal ~half the
                # bandwidth from the critical stream (measured: A1
                # starved when mw/slab2 ran on the scalar queue early).
                # Chunking wv0/slab0 lets the first ps1 matmul wait on
                # only ~540KB instead of ~1.4MB.
                kslab_tiles = [
                    slabs.tile([P, 4, DC, P], FP16, tag="slab", name=f"kslab{s}")
                    for s in range(4)
                ]
                nc.sync.dma_start(
                    out=wv_sb[:, 0, 0:4], in_=wvi[:, 0 : 4 * 384]
                )
                nc.sync.dma_start(
                    out=kslab_tiles[0][:, 0:2], in_=xh[0][:, 0 : 2 * DC * P]
                )
                nc.sync.dma_start(
                    out=wv_sb[:, 0, 4:6], in_=wvi[:, 4 * 384 : DC * 384]
                )
                nc.sync.dma_start(
                    out=kslab_tiles[0][:, 2:4], in_=xh[0][:, 2 * DC * P :]
                )
                nc.sync.dma_start(
                    out=kslab_tiles[1][:, 0:2], in_=xh[1][:, 0 : 2 * DC * P]
                )
                nc.sync.dma_start(
                    out=kslab_tiles[1][:, 2:4], in_=xh[1][:, 2 * DC * P :]
                )
                nc.sync.dma_start(
                    out=wv_sb[:, 1], in_=wvi[:, DC * 384 :]
                )
                nc.sync.dma_start(out=mw_sb, in_=mw[:, :])
                nc.sync.dma_start(out=kslab_tiles[3], in_=xh[3])
                nc.sync.dma_start(out=kslab_tiles[2], in_=xh[2])

                # ---- A1: V' own half (earliest -> feeds the gather) ----
                # kc0/kc1 run their wv0a-covered dc 0:4 halves FIRST
                # (two open psum groups on different banks): kc1's dc0-3
                # fills the ~1us wait for the wv0b chunk (dc 4-5) that a
                # straight kc0 dc-loop exposes right at kernel start.
                ps_first = [
                    psa.tile([P, 512], F32, tag="psa", name=f"ps1f{k}")
                    for k in range(2)
                ]
                for k in range(2):
                    for dc in range(4):
                        nc.tensor.matmul(
                            ps_first[k][:, 0:384],
                            kslab_tiles[0][:, k, dc, :],
                            wv_sb[:, 0, dc, :],
                            start=(dc == 0),
                            stop=False,
                        )
                for k in range(2):
                    for dc in range(4, DC):
                        nc.tensor.matmul(
                            ps_first[k][:, 0:384],
                            kslab_tiles[0][:, k, dc, :],
                            wv_sb[:, 0, dc, :],
                            start=False,
                            stop=(dc == DC - 1),
                        )
                    nc.vector.tensor_copy(
                        vpA[:, k, 0:384], ps_first[k][:, 0:384]
                    )
                for kc in range(2, HKC):
                    slab = kslab_tiles[kc // 4]
                    ps1 = psa.tile([P, 512], F32, tag="psa")
                    for dc in range(DC):
                        nc.tensor.matmul(
                            ps1[:, 0:384],
                            slab[:, kc % 4, dc, :],
                            wv_sb[:, 0, dc, :],
                            start=(dc == 0),
                            stop=(dc == DC - 1),
                        )
                    nc.vector.tensor_copy(vpA[:, kc, 0:384], ps1[:, 0:384])
                for kc in range(HKC):
                    slab = kslab_tiles[kc // 4]
                    ps2 = psa.tile([P, 512], F32, tag="psa")
                    for dc in range(DC):
                        nc.tensor.matmul(
                            ps2[:, 0:384],
                            slab[:, kc % 4, dc, :],
                            wv_sb[:, 1, dc, :],
                            start=(dc == 0),
                            stop=(dc == DC - 1),
                        )
                    nc.vector.tensor_copy(vpA[:, kc, 384:OUT], ps2[:, 0:384])
                    nc.vector.tensor_copy(vpA[:, kc, OUT : OUT + 1], ones_sc)
                    nc.vector.tensor_copy(
                        vpA[:, kc, OUT + 1 : OUT + 2], zero_sc
                    )
                    nc.gpsimd.dma_start(
                        out=vpb_in[:, kc, :], in_=vpA[:, kc, :]
                    )
                nc.gpsimd.collective_compute(
                    "AllGather",
                    mybir.AluOpType.bypass,
                    replica_groups=PAIRS,
                    ins=[vpb_in.opt()],
                    outs=[vpb_out.opt()],
                )
                # Peer-half readback: the gather output is rank-ordered,
                # so rank r's peer sits in slot 1-r. Two complementary
                # predicated DMAs keep the APs static; the skipped DMA
                # still increments the semaphore, so downstream waits
                # count identically on both ranks. Both on the sync
                # queue (single-queue writers keep the matmul wait).
                me = nc.sync.partition_id() % 2
                nc.sync.dma_start(out=vpB[:], in_=vpb_out[0], cond=me)
                nc.sync.dma_start(
                    out=vpB[:], in_=vpb_out[1], cond=(me + 1) % 2
                )

                # ---- A2: Q'^T = (x_q M)^T own half ----
                for s in range(2):
                    slab = kslab_tiles[s]
                    for oc in range(DC):
                        ps = psa.tile([P, 512], F32, tag="psa")
                        for dc in range(DC):
                            nc.tensor.matmul(
                                ps,
                                mw_sb[:, dc, oc * P : (oc + 1) * P],
                                slab[:, :, dc, :],
                                start=(dc == 0),
                                stop=(dc == DC - 1),
                            )
                        nc.vector.tensor_copy(qpt[s][:, oc, :], ps)

                # ---- scoresT: contracts over d, stationary = xh slab
                # chunks (rank-relative key order), moving = Q'^T. Runs
                # inside the psa scope (ps_sc has its own banks) so no
                # pool-close fence sits between A2 and scores.
                for bi in range(2):
                    for kc in range(KC):
                        kslab = kslab_tiles[kc // 4]
                        st = ps_sc.tile([P, 512], F32, tag="sc")
                        for dc in range(DC):
                            nc.tensor.matmul(
                                st,
                                kslab[:, kc % 4, dc, :],
                                qpt[bi][:, dc, :],
                                start=(dc == 0),
                                stop=(dc == DC - 1),
                            )
                        et = expp.tile(
                            [P, 512], FP16, tag="exp", name=f"et{bi}_{kc}"
                        )
                        nc.scalar.activation(
                            et,
                            st,
                            mybir.ActivationFunctionType.Exp,
                            scale=0.125 / M_SCALE,
                        )
                        ets[(bi, kc)] = et

            # ---- out phase: psa's banks freed above feed ps_out; the
            # pool-close fence overlaps the V-gather wait.
            with (
                tc.tile_pool(name="ps_den", bufs=3, space="PSUM") as ps_den,
                tc.tile_pool(name="ps_av", bufs=3, space="PSUM") as ps_av,
            ):
                # 8 q-chunks of 128. Denominator run (cols 512:770) goes
                # FIRST into its OWN 1-bank psum tile so the recip and
                # 512:768 normalize never WAR-block the 0:512 run (a
                # shared tile serialized run2 behind the normalize —
                # measured ~0.7us per exposed boundary); kc 0-7 read vpA
                # (local), kc 8-15 read vpB (gathered peer half).
                def vsrc(kc):
                    return vpA if kc < HKC else vpB

                for j in range(NQ // P):
                    bi, jj = j // 4, j % 4
                    opd = ps_den.tile([P, 258], F32, tag="den", name=f"den{j}")
                    for kc in range(KC):
                        nc.tensor.matmul(
                            opd,
                            ets[(bi, kc)][:, jj * P : (jj + 1) * P],
                            vsrc(kc)[:, kc % HKC, 512 : OUT + 2],
                            start=(kc == 0),
                            stop=(kc == KC - 1),
                        )
                    recip = smallp.tile([P, 1], F32, tag="recip")
                    nc.vector.reciprocal(recip, opd[:, 256:257])
                    ob = obp.tile([P, OUT], FP16, tag="ob")
                    nc.vector.tensor_scalar_mul(
                        ob[:, 512:OUT], opd[:, 0:256], recip
                    )
                    if j == NQ // P - 1:
                        # last chunk: ship cols 512:768 now (scalar
                        # queue), run the 0:512 accumulation as two
                        # 256-col runs in separate psum tiles so the
                        # first half's normalize + DMA overlap the
                        # second half's matmuls.
                        nc.scalar.dma_start(
                            out=out[j * P : (j + 1) * P, 512:OUT],
                            in_=ob[:, 512:OUT],
                        )
                        # the last piece rides the scalar queue DIRECTLY
                        # behind the 256:448 piece: a busy queue picks up
                        # the next descriptor without the ~1us idle-queue
                        # doorbell latency the sync queue would pay.
                        for lo, hi, eng in (
                            (0, 256, nc.sync),
                            (256, 448, nc.scalar),
                            (448, 512, nc.scalar),
                        ):
                            opa = ps_av.tile(
                                [P, hi - lo], F32, tag="av", name=f"av{j}_{lo}"
                            )
                            for kc in range(KC):
                                nc.tensor.matmul(
                                    opa,
                                    ets[(bi, kc)][:, jj * P : (jj + 1) * P],
                                    vsrc(kc)[:, kc % HKC, lo:hi],
                                    start=(kc == 0),
                                    stop=(kc == KC - 1),
                                )
                            nc.vector.tensor_scalar_mul(
                                ob[:, lo:hi], opa, recip
                            )
                            eng.dma_start(
                                out=out[j * P : (j + 1) * P, lo:hi],
                                in_=ob[:, lo:hi],
                            )
                    else:
                        opa = ps_av.tile([P, 512], F32, tag="av", name=f"av{j}")
                        for kc in range(KC):
                            nc.tensor.matmul(
                                opa,
                                ets[(bi, kc)][:, jj * P : (jj + 1) * P],
                                vsrc(kc)[:, kc % HKC, 0:512],
                                start=(kc == 0),
                                stop=(kc == KC - 1),
                            )
                        nc.vector.tensor_scalar_mul(
                            ob[:, 0:512], opa, recip
                        )
                        nc.sync.dma_start(
                            out=out[j * P : (j + 1) * P, :], in_=ob
                        )
    nc.finalize()
    return nc


_NC_CACHE = None


def _get_nc():
    global _NC_CACHE
    if _NC_CACHE is None:
        _NC_CACHE = build_attention_nc()
    return _NC_CACHE


def _xh_layout(a2d):
    """[D, 2048] -> [4, P, 4*DC*128], quarter-major slabs: the kc-th
    128-token quarter of a slab is a contiguous DMA prefix."""
    t = a2d.reshape(DC, P, 4, 4, P)  # dc p s q t
    t = t.transpose(2, 1, 3, 0, 4)  # s p q dc t
    return np.ascontiguousarray(t.reshape(4, P, 4 * DC * P))


def _wv_layout(a2d):
    """[D, 768] -> [P, 2*DC*384], column-half-major."""
    t = a2d.reshape(DC, P, 2, 384)  # dc p h c
    t = t.transpose(1, 2, 0, 3)  # p h dc c
    return np.ascontiguousarray(t.reshape(P, 2 * DC * 384))


def _mw_layout(a2d):
    """[D, D] -> [P, DC*D], dc-major."""
    t = a2d.reshape(DC, P, D).transpose(1, 0, 2)
    return np.ascontiguousarray(t.reshape(P, DC * D))


def make_in_maps(x, kernel):
    x = np.asarray(x, dtype=np.float32)
    w = np.asarray(kernel, dtype=np.float32)
    mw16 = (M_SCALE * (w[0] @ w[1].T)).astype(np.float16)
    mw = _mw_layout(mw16)
    wv = _wv_layout(w[2].astype(np.float16))
    in_maps = []
    for core in range(N_CORES):
        b, half = core // 2, core % 2
        xt16 = x[b].T.astype(np.float16)
        # rank-relative key order: own 1024 tokens first, then peer's
        own = xt16[:, half * NQ : (half + 1) * NQ]
        peer = xt16[:, (1 - half) * NQ : (2 - half) * NQ]
        xh = _xh_layout(np.concatenate([own, peer], axis=1))
        in_maps.append({"xh": xh, "mw": mw, "wvi": wv})
    return in_maps


def assemble_output(results):
    out = np.empty((B, N, OUT), dtype=np.float32)
    for core in range(N_CORES):
        b, half = core // 2, core % 2
        out[b, half * NQ : (half + 1) * NQ, :] = results[core]["out"]
    return out


def run_on_hw(x, kernel, trace=False):
    nc = _get_nc()
    res = run_bass_kernel_spmd(
        nc, make_in_maps(x, kernel), list(range(N_CORES)), trace=trace
    )
    return assemble_output(res.results), res


def kernel(x, kernel):
    out, _ = run_on_hw(x, kernel, trace=False)
    return out



# revision 35
# speedup vs baseline: 1.1019x; 1.0028x over previous
"""Attention with host-folded QK^T kernel + pair-wise V dedup AllGather.

v12: the K projection never runs on device. scores = q·k^T with
q = x W_q, k = x W_k factors as x (W_q W_k^T) x^T, so the host
precomputes M = 64·W_q W_k^T (fp32 matmul, then fp16 — the 64×
scale keeps M's ~1e-5-magnitude entries out of fp16 subnormals;
the exp activation scale absorbs the 1/64). Each core computes
Q' = x_q M for its query half (same cost as the old Q projection)
and scores come from Q'·x_k^T against the xh slabs directly —
the entire 32µs redundant full-K projection is gone.

Keys are RANK-RELATIVE: the host builds each core's xh as
[own 1024 tokens | peer 1024 tokens], which (a) makes xq redundant
(A1/A2 read the first two xh slabs), and (b) lets V' for the own
half live entirely on-core (vpA) — only the PEER half of V' comes
back from the 2-rank AllGather. The gather output is rank-ordered,
so the peer slot index depends on the rank; two complementary
cond= predicated DMAs (skipped DMAs still increment their
semaphore) funnel the right slot into vpB with fully static APs.
Attention is an order-free reduction over keys, so rank-relative
key order changes nothing downstream.

Schedule notes (v13):
- ALL input DMAs ride ONE hardware queue (sync) in strict need
  order: the 16 DMA engines are shared across queues, so a
  second concurrent queue steals ~half the bandwidth from the
  critical stream (measured). Chunked wv0/slab0/slab1 keep the
  per-DMA completion semaphores fine-grained, so the first ps1
  matmul waits on ~690KB, not 1.4MB, and ps1-kc4 waits on half
  of slab1, not all of it.
- 10 HAM-warmup matmuls (memset on vector) bridge the ~5.5us
  DMA wait and un-throttle the PE clock gate (K=4/8 1.2GHz ->
  K=8/8 2.4GHz after ~3.4us of sustained activity), so the first
  REAL matmul already runs at 2.4GHz (one K=8 HAM event spans
  the whole kernel in the trace).
- A1 kc0/kc1 run their wv0a-covered dc halves first as two open
  psum groups: kc1's dc0-3 fills the ~1.1us wait for the wv0b
  chunk. With this the whole work span measures within ~0.3us of
  the 270592-column streaming ideal (zero stalls >160ns).
- the scalar DMA queue is pre-warmed with a tiny transfer; its
  only other use is the final output chunks, which otherwise pay
  the ~1.6us first-doorbell queue-start latency on the tail.
- ps_sc is allocated OUTSIDE the psa scope and the scores loop
  runs INSIDE it, so no pool-close fence sits between A2 and
  scores (measured 1.05us of PE idle otherwise); the psa close
  lands at the scores->out boundary instead.
- PSUM budget: wps(1) + psa(5) + ps_sc(2) = 8 banks in phase A;
  out phase: ps_den(3x1) + ps_av(3x1) reuse psa's banks.
- out phase: the denominator run (vp cols 512:770) accumulates
  into its OWN 1-bank psum tile (ps_den). With a shared tile the
  0:512 run was WAR-serialized behind the vector normalize that
  reads the denominator half (~0.7us per exposed boundary; Tile
  tracks hazards at tile granularity).
- vpA and vpB are separate tiles: a single tile written by two
  different DMA queues loses one of the matmul waits (HW 1-wait
  limit) — observed as a cold-run race.
- last q-chunk: cols 512:768 ship right after the denominator
  normalize; the 0:512 run is split 256/192/64 with interleaved
  normalize+DMA on alternating warm queues, so only a 64-col DMA
  and one small normalize sit after the final matmul (~1.9us
  tail vs ~3.8us for a monolithic last chunk).
- the output tensor is fp16 (host upcasts to fp32): halves the
  output DMA bytes on the tail and doubles the DVE normalize
  rate; adds ~1e-4 quantization (rel err 8.0e-4 -> 9.1e-4).
- fp8 DoubleRow was evaluated and rejected: e4m3 on any of the
  big matmuls gives 3.3e-2..5.7e-2 max-rel error (near-one-hot
  softmax rows pass single-element quantization error straight
  through) vs the 2e-2 gate.
"""

import numpy as np

import concourse.bass as bass
import concourse.mybir as mybir
import concourse.tile as tile
from concourse import bacc
from concourse.bass_utils import run_bass_kernel_spmd

N_CORES = 8
B, N, D, OUT = 4, 2048, 768, 768
NQ = N // 2
P = 128
DC = D // P
KC = N // P
HKC = KC // 2  # k-chunks per half
F32 = mybir.dt.float32
FP16 = mybir.dt.float16
PAIRS = [[0, 1], [2, 3], [4, 5], [6, 7]]

M_SCALE = 64.0  # host folds this into M; exp scale divides it back out


def build_attention_nc():
    nc = bacc.Bacc("TRN2", target_bir_lowering=False, debug=False)
    # Inputs host-pre-arranged in SBUF slab layout [p, dc, n]; xh is
    # rank-relative: slabs 0-1 = own 1024 tokens, 2-3 = peer tokens.
    xh = nc.dram_tensor("xh", [4, P, DC * 512], FP16, kind="ExternalInput")
    mw = nc.dram_tensor("mw", [P, DC * D], FP16, kind="ExternalInput")
    wvi = nc.dram_tensor("wvi", [P, 2 * DC * 384], FP16, kind="ExternalInput")
    # fp16 output: halves the output DMA bytes (the final chunk sits on
    # the critical tail) and doubles the DVE normalize rate; the ~5e-4
    # fp16 quantization is negligible vs the 2e-2 gate (host upcasts).
    out = nc.dram_tensor("out", [NQ, OUT], FP16, kind="ExternalOutput")

    with tile.TileContext(nc) as tc:
        with (
            tc.tile_pool(name="persist", bufs=1) as persist,
            tc.tile_pool(name="slabs", bufs=4) as slabs,
            tc.tile_pool(name="wpool", bufs=1) as wpool,
            tc.tile_pool(name="expp", bufs=34) as expp,
            tc.tile_pool(name="obp", bufs=3) as obp,
            tc.tile_pool(name="smallp", bufs=4) as smallp,
            tc.tile_pool(name="ps_sc", bufs=2, space="PSUM") as ps_sc,
            tc.tile_pool(name="dpool", bufs=1, space="DRAM") as dpool,
        ):
            # Q'^T[d,q], one tile per 512-query half so the scores phase
            # never waits on the other half's psum drain
            qpt = [
                persist.tile([P, DC, 512], FP16, name=f"qpt{s}")
                for s in range(2)
            ]
            # V' in rank-relative key order: vpA = own half (local only),
            # vpB = peer half (from the gather)
            vpA = persist.tile([P, HKC, OUT + 2], FP16, name="vpA")
            vpB = persist.tile([P, HKC, OUT + 2], FP16, name="vpB")

            vpb_in = dpool.tile([P, HKC, OUT + 2], FP16)
            vpb_out = dpool.tile([2, P, HKC, OUT + 2], FP16)

            wv_sb = wpool.tile([P, 2, DC, 384], FP16)
            mw_sb = wpool.tile([P, DC, D], FP16)

            # HAM warmup; memset on vector (earliest-idle engine) so the
            # ramp matmuls can start right after the preamble barrier.
            warm = wpool.tile([P, 512], FP16, name="warm")
            nc.vector.memset(warm, 1.0)

            # Pre-warm the scalar DMA queue with a tiny transfer: its
            # only other use is the final output chunk, which otherwise
            # pays the ~1.6us first-doorbell queue-start latency right
            # on the critical tail.
            qwarm = wpool.tile([P, 8], FP16, name="qwarm")
            nc.scalar.dma_start(out=qwarm, in_=mw[:, 0:8])

            ones_sc = persist.tile([P, 1], F32, name="ones_sc")
            nc.vector.memset(ones_sc, 1.0)
            zero_sc = persist.tile([P, 1], F32, name="zero_sc")
            nc.vector.memset(zero_sc, 0.0)

            ets = {}
            with tc.tile_pool(name="psa", bufs=5, space="PSUM") as psa:
                wps = psa.tile([P, 512], F32, name="wps", bufs=1)
                for i in range(10):
                    nc.tensor.matmul(
                        wps, warm[:, 0:P], warm, start=(i == 0), stop=(i == 9)
                    )

                # All input DMAs ride ONE queue (sync) in strict NEED
                # order: the 16 DMA engines are shared across queues, so
                # a second concurrent queue would steal ~half the
                # bandwidth from the critical stream (measured: A1
                # starved when mw/slab2 ran on the scalar queue early).
                # Chunking wv0/slab0 lets the first ps1 matmul wait on
                # only ~540KB instead of ~1.4MB.
                kslab_tiles = [
                    slabs.tile([P, 4, DC, P], FP16, tag="slab", name=f"kslab{s}")
                    for s in range(4)
                ]
                nc.sync.dma_start(
                    out=wv_sb[:, 0, 0:4], in_=wvi[:, 0 : 4 * 384]
                )
                nc.sync.dma_start(
                    out=kslab_tiles[0][:, 0:2], in_=xh[0][:, 0 : 2 * DC * P]
                )
                nc.sync.dma_start(
                    out=wv_sb[:, 0, 4:6], in_=wvi[:, 4 * 384 : DC * 384]
                )
                nc.sync.dma_start(
                    out=kslab_tiles[0][:, 2:4], in_=xh[0][:, 2 * DC * P :]
                )
                nc.sync.dma_start(
                    out=kslab_tiles[1][:, 0:2], in_=xh[1][:, 0 : 2 * DC * P]
                )
                nc.sync.dma_start(
                    out=kslab_tiles[1][:, 2:4], in_=xh[1][:, 2 * DC * P :]
                )
                nc.sync.dma_start(
                    out=wv_sb[:, 1], in_=wvi[:, DC * 384 :]
                )
                nc.sync.dma_start(out=mw_sb, in_=mw[:, :])
                nc.sync.dma_start(out=kslab_tiles[3], in_=xh[3])
                nc.sync.dma_start(out=kslab_tiles[2], in_=xh[2])

                # ---- A1: V' own half (earliest -> feeds the gather) ----
                # kc0/kc1 run their wv0a-covered dc 0:4 halves FIRST
                # (two open psum groups on different banks): kc1's dc0-3
                # fills the ~1us wait for the wv0b chunk (dc 4-5) that a
                # straight kc0 dc-loop exposes right at kernel start.
                ps_first = [
                    psa.tile([P, 512], F32, tag="psa", name=f"ps1f{k}")
                    for k in range(2)
                ]
                for k in range(2):
                    for dc in range(4):
                        nc.tensor.matmul(
                            ps_first[k][:, 0:384],
                            kslab_tiles[0][:, k, dc, :],
                            wv_sb[:, 0, dc, :],
                            start=(dc == 0),
                            stop=False,
                        )
                for k in range(2):
                    for dc in range(4, DC):
                        nc.tensor.matmul(
                            ps_first[k][:, 0:384],
                            kslab_tiles[0][:, k, dc, :],
                            wv_sb[:, 0, dc, :],
                            start=False,
                            stop=(dc == DC - 1),
                        )
                    nc.vector.tensor_copy(
                        vpA[:, k, 0:384], ps_first[k][:, 0:384]
                    )
                for kc in range(2, HKC):
                    slab = kslab_tiles[kc // 4]
                    ps1 = psa.tile([P, 512], F32, tag="psa")
                    for dc in range(DC):
                        nc.tensor.matmul(
                            ps1[:, 0:384],
                            slab[:, kc % 4, dc, :],
                            wv_sb[:, 0, dc, :],
                            start=(dc == 0),
                            stop=(dc == DC - 1),
                        )
                    nc.vector.tensor_copy(vpA[:, kc, 0:384], ps1[:, 0:384])
                for kc in range(HKC):
                    slab = kslab_tiles[kc // 4]
                    ps2 = psa.tile([P, 512], F32, tag="psa")
                    for dc in range(DC):
                        nc.tensor.matmul(
                            ps2[:, 0:384],
                            slab[:, kc % 4, dc, :],
                            wv_sb[:, 1, dc, :],
                            start=(dc == 0),
                            stop=(dc == DC - 1),
                        )
                    nc.vector.tensor_copy(vpA[:, kc, 384:OUT], ps2[:, 0:384])
                    nc.vector.tensor_copy(vpA[:, kc, OUT : OUT + 1], ones_sc)
                    nc.vector.tensor_copy(
                        vpA[:, kc, OUT + 1 : OUT + 2], zero_sc
                    )
                    nc.gpsimd.dma_start(
                        out=vpb_in[:, kc, :], in_=vpA[:, kc, :]
                    )
                nc.gpsimd.collective_compute(
                    "AllGather",
                    mybir.AluOpType.bypass,
                    replica_groups=PAIRS,
                    ins=[vpb_in.opt()],
                    outs=[vpb_out.opt()],
                )
                # Peer-half readback: the gather output is rank-ordered,
                # so rank r's peer sits in slot 1-r. Two complementary
                # predicated DMAs keep the APs static; the skipped DMA
                # still increments the semaphore, so downstream waits
                # count identically on both ranks. Both on the sync
                # queue (single-queue writers keep the matmul wait).
                me = nc.sync.partition_id() % 2
                nc.sync.dma_start(out=vpB[:], in_=vpb_out[0], cond=me)
                nc.sync.dma_start(
                    out=vpB[:], in_=vpb_out[1], cond=(me + 1) % 2
                )

                # ---- A2: Q'^T = (x_q M)^T own half ----
                for s in range(2):
                    slab = kslab_tiles[s]
                    for oc in range(DC):
                        ps = psa.tile([P, 512], F32, tag="psa")
                        for dc in range(DC):
                            nc.tensor.matmul(
                                ps,
                                mw_sb[:, dc, oc * P : (oc + 1) * P],
                                slab[:, :, dc, :],
                                start=(dc == 0),
                                stop=(dc == DC - 1),
                            )
                        nc.vector.tensor_copy(qpt[s][:, oc, :], ps)

                # ---- scoresT: contracts over d, stationary = xh slab
                # chunks (rank-relative key order), moving = Q'^T. Runs
                # inside the psa scope (ps_sc has its own banks) so no
                # pool-close fence sits between A2 and scores.
                for bi in range(2):
                    for kc in range(KC):
                        kslab = kslab_tiles[kc // 4]
                        st = ps_sc.tile([P, 512], F32, tag="sc")
                        for dc in range(DC):
                            nc.tensor.matmul(
                                st,
                                kslab[:, kc % 4, dc, :],
                                qpt[bi][:, dc, :],
                                start=(dc == 0),
                                stop=(dc == DC - 1),
                            )
                        et = expp.tile(
                            [P, 512], FP16, tag="exp", name=f"et{bi}_{kc}"
                        )
                        nc.scalar.activation(
                            et,
                            st,
                            mybir.ActivationFunctionType.Exp,
                            scale=0.125 / M_SCALE,
                        )
                        ets[(bi, kc)] = et

            # ---- out phase: psa's banks freed above feed ps_out; the
            # pool-close fence overlaps the V-gather wait.
            with (
                tc.tile_pool(name="ps_den", bufs=3, space="PSUM") as ps_den,
                tc.tile_pool(name="ps_av", bufs=3, space="PSUM") as ps_av,
            ):
                # 8 q-chunks of 128. Denominator run (cols 512:770) goes
                # FIRST into its OWN 1-bank psum tile so the recip and
                # 512:768 normalize never WAR-block the 0:512 run (a
                # shared tile serialized run2 behind the normalize —
                # measured ~0.7us per exposed boundary); kc 0-7 read vpA
                # (local), kc 8-15 read vpB (gathered peer half).
                def vsrc(kc):
                    return vpA if kc < HKC else vpB

                for j in range(NQ // P):
                    bi, jj = j // 4, j % 4
                    opd = ps_den.tile([P, 258], F32, tag="den", name=f"den{j}")
                    for kc in range(KC):
                        nc.tensor.matmul(
                            opd,
                            ets[(bi, kc)][:, jj * P : (jj + 1) * P],
                            vsrc(kc)[:, kc % HKC, 512 : OUT + 2],
                            start=(kc == 0),
                            stop=(kc == KC - 1),
                        )
                    recip = smallp.tile([P, 1], F32, tag="recip")
                    nc.vector.reciprocal(recip, opd[:, 256:257])
                    ob = obp.tile([P, OUT], FP16, tag="ob")
                    nc.vector.tensor_scalar_mul(
                        ob[:, 512:OUT], opd[:, 0:256], recip
                    )
                    if j == NQ // P - 1:
                        # last chunk: ship cols 512:768 now (scalar
                        # queue), run the 0:512 accumulation as two
                        # 256-col runs in separate psum tiles so the
                        # first half's normalize + DMA overlap the
                        # second half's matmuls.
                        nc.scalar.dma_start(
                            out=out[j * P : (j + 1) * P, 512:OUT],
                            in_=ob[:, 512:OUT],
                        )
                        for lo, hi, eng in (
                            (0, 256, nc.sync),
                            (256, 448, nc.scalar),
                            (448, 512, nc.sync),
                        ):
                            opa = ps_av.tile(
                                [P, hi - lo], F32, tag="av", name=f"av{j}_{lo}"
                            )
                            for kc in range(KC):
                                nc.tensor.matmul(
                                    opa,
                                    ets[(bi, kc)][:, jj * P : (jj + 1) * P],
                                    vsrc(kc)[:, kc % HKC, lo:hi],
                                    start=(kc == 0),
                                    stop=(kc == KC - 1),
                                )
                            nc.vector.tensor_scalar_mul(
                                ob[:, lo:hi], opa, recip
                            )
                            eng.dma_start(
                                out=out[j * P : (j + 1) * P, lo:hi],
                                in_=ob[:, lo:hi],
                            )
                    else:
                        opa = ps_av.tile([P, 512], F32, tag="av", name=f"av{j}")
                        for kc in range(KC):
                            nc.tensor.matmul(
                                opa,
                                ets[(bi, kc)][:, jj * P : (jj + 1) * P],
                                vsrc(kc)[:, kc % HKC, 0:512],
                                start=(kc == 0),
                                stop=(kc == KC - 1),
                            )
                        nc.vector.tensor_scalar_mul(
                            ob[:, 0:512], opa, recip
                        )
                        nc.sync.dma_start(
                            out=out[j * P : (j + 1) * P, :], in_=ob
                        )
    nc.finalize()
    return nc


_NC_CACHE = None


def _get_nc():
    global _NC_CACHE
    if _NC_CACHE is None:
        _NC_CACHE = build_attention_nc()
    return _NC_CACHE


def _xh_layout(a2d):
    """[D, 2048] -> [4, P, 4*DC*128], quarter-major slabs: the kc-th
    128-token quarter of a slab is a contiguous DMA prefix."""
    t = a2d.reshape(DC, P, 4, 4, P)  # dc p s q t
    t = t.transpose(2, 1, 3, 0, 4)  # s p q dc t
    return np.ascontiguousarray(t.reshape(4, P, 4 * DC * P))


def _wv_layout(a2d):
    """[D, 768] -> [P, 2*DC*384], column-half-major."""
    t = a2d.reshape(DC, P, 2, 384)  # dc p h c
    t = t.transpose(1, 2, 0, 3)  # p h dc c
    return np.ascontiguousarray(t.reshape(P, 2 * DC * 384))


def _mw_layout(a2d):
    """[D, D] -> [P, DC*D], dc-major."""
    t = a2d.reshape(DC, P, D).transpose(1, 0, 2)
    return np.ascontiguousarray(t.reshape(P, DC * D))


def make_in_maps(x, kernel):
    x = np.asarray(x, dtype=np.float32)
    w = np.asarray(kernel, dtype=np.float32)
    mw16 = (M_SCALE * (w[0] @ w[1].T)).astype(np.float16)
    mw = _mw_layout(mw16)
    wv = _wv_layout(w[2].astype(np.float16))
    in_maps = []
    for core in range(N_CORES):
        b, half = core // 2, core % 2
        xt16 = x[b].T.astype(np.float16)
        # rank-relative key order: own 1024 tokens first, then peer's
        own = xt16[:, half * NQ : (half + 1) * NQ]
        peer = xt16[:, (1 - half) * NQ : (2 - half) * NQ]
        xh = _xh_layout(np.concatenate([own, peer], axis=1))
        in_maps.append({"xh": xh, "mw": mw, "wvi": wv})
    return in_maps


def assemble_output(results):
    out = np.empty((B, N, OUT), dtype=np.float32)
    for core in range(N_CORES):
        b, half = core // 2, core % 2
        out[b, half * NQ : (half + 1) * NQ, :] = results[core]["out"]
    return out


def run_on_hw(x, kernel, trace=False):
    nc = _get_nc()
    res = run_bass_kernel_spmd(
        nc, make_in_maps(x, kernel), list(range(N_CORES)), trace=trace
    )
    return assemble_output(res.results), res


def kernel(x, kernel):
    out, _ = run_on_hw(x, kernel, trace=False)
    return out



# revision 36
# speedup vs baseline: 1.1090x; 1.0064x over previous
"""Attention with host-folded QK^T kernel + pair-wise V dedup AllGather.

v12: the K projection never runs on device. scores = q·k^T with
q = x W_q, k = x W_k factors as x (W_q W_k^T) x^T, so the host
precomputes M = 64·W_q W_k^T (fp32 matmul, then fp16 — the 64×
scale keeps M's ~1e-5-magnitude entries out of fp16 subnormals;
the exp activation scale absorbs the 1/64). Each core computes
Q' = x_q M for its query half (same cost as the old Q projection)
and scores come from Q'·x_k^T against the xh slabs directly —
the entire 32µs redundant full-K projection is gone.

Keys are RANK-RELATIVE: the host builds each core's xh as
[own 1024 tokens | peer 1024 tokens], which (a) makes xq redundant
(A1/A2 read the first two xh slabs), and (b) lets V' for the own
half live entirely on-core (vpA) — only the PEER half of V' comes
back from the 2-rank AllGather. The gather output is rank-ordered,
so the peer slot index depends on the rank; two complementary
cond= predicated DMAs (skipped DMAs still increment their
semaphore) funnel the right slot into vpB with fully static APs.
Attention is an order-free reduction over keys, so rank-relative
key order changes nothing downstream.

Schedule notes (v13):
- ALL input DMAs ride ONE hardware queue (sync) in strict need
  order: the 16 DMA engines are shared across queues, so a
  second concurrent queue steals ~half the bandwidth from the
  critical stream (measured). Chunked wv0/slab0/slab1 keep the
  per-DMA completion semaphores fine-grained, so the first ps1
  matmul waits on ~690KB, not 1.4MB, and ps1-kc4 waits on half
  of slab1, not all of it.
- 10 HAM-warmup matmuls (memset on vector) bridge the ~5.5us
  DMA wait and un-throttle the PE clock gate (K=4/8 1.2GHz ->
  K=8/8 2.4GHz after ~3.4us of sustained activity), so the first
  REAL matmul already runs at 2.4GHz (one K=8 HAM event spans
  the whole kernel in the trace).
- A1 kc0/kc1 run their wv0a-covered dc halves first as two open
  psum groups: kc1's dc0-3 fills the ~1.1us wait for the wv0b
  chunk. With this the whole work span measures within ~0.3us of
  the 270592-column streaming ideal (zero stalls >160ns).
- the scalar DMA queue is pre-warmed with a tiny transfer; its
  only other use is the final output chunks, which otherwise pay
  the ~1.6us first-doorbell queue-start latency on the tail.
- ps_sc is allocated OUTSIDE the psa scope and the scores loop
  runs INSIDE it, so no pool-close fence sits between A2 and
  scores (measured 1.05us of PE idle otherwise); the psa close
  lands at the scores->out boundary instead.
- PSUM budget: wps(1) + psa(5) + ps_sc(2) = 8 banks in phase A;
  out phase: ps_den(3x1) + ps_av(3x1) reuse psa's banks.
- out phase: the denominator run (vp cols 512:770) accumulates
  into its OWN 1-bank psum tile (ps_den). With a shared tile the
  0:512 run was WAR-serialized behind the vector normalize that
  reads the denominator half (~0.7us per exposed boundary; Tile
  tracks hazards at tile granularity).
- vpA and vpB are separate tiles: a single tile written by two
  different DMA queues loses one of the matmul waits (HW 1-wait
  limit) — observed as a cold-run race.
- last q-chunk: cols 512:768 ship right after the denominator
  normalize; the 0:512 run is split 256/192/64 with interleaved
  normalize+DMA on alternating warm queues, so only a 64-col DMA
  and one small normalize sit after the final matmul (~1.9us
  tail vs ~3.8us for a monolithic last chunk).
- the output tensor is fp16 (host upcasts to fp32): halves the
  output DMA bytes on the tail and doubles the DVE normalize
  rate; adds ~1e-4 quantization (rel err 8.0e-4 -> 9.1e-4).
- fp8 DoubleRow was evaluated and rejected: e4m3 on any of the
  big matmuls gives 3.3e-2..5.7e-2 max-rel error (near-one-hot
  softmax rows pass single-element quantization error straight
  through) vs the 2e-2 gate.
"""

import numpy as np

import concourse.bass as bass
import concourse.mybir as mybir
import concourse.tile as tile
from concourse import bacc
from concourse.bass_utils import run_bass_kernel_spmd

N_CORES = 8
B, N, D, OUT = 4, 2048, 768, 768
NQ = N // 2
P = 128
DC = D // P
KC = N // P
HKC = KC // 2  # k-chunks per half
F32 = mybir.dt.float32
FP16 = mybir.dt.float16
PAIRS = [[0, 1], [2, 3], [4, 5], [6, 7]]

M_SCALE = 64.0  # host folds this into M; exp scale divides it back out


def build_attention_nc():
    nc = bacc.Bacc("TRN2", target_bir_lowering=False, debug=False)
    # Inputs host-pre-arranged in SBUF slab layout [p, dc, n]; xh is
    # rank-relative: slabs 0-1 = own 1024 tokens, 2-3 = peer tokens.
    xh = nc.dram_tensor("xh", [4, P, DC * 512], FP16, kind="ExternalInput")
    mw = nc.dram_tensor("mw", [P, DC * D], FP16, kind="ExternalInput")
    wvi = nc.dram_tensor("wvi", [P, 2 * DC * 384], FP16, kind="ExternalInput")
    # fp16 output: halves the output DMA bytes (the final chunk sits on
    # the critical tail) and doubles the DVE normalize rate; the ~5e-4
    # fp16 quantization is negligible vs the 2e-2 gate (host upcasts).
    out = nc.dram_tensor("out", [NQ, OUT], FP16, kind="ExternalOutput")

    with tile.TileContext(nc) as tc:
        with (
            tc.tile_pool(name="persist", bufs=1) as persist,
            tc.tile_pool(name="slabs", bufs=4) as slabs,
            tc.tile_pool(name="wpool", bufs=1) as wpool,
            tc.tile_pool(name="expp", bufs=34) as expp,
            tc.tile_pool(name="obp", bufs=3) as obp,
            tc.tile_pool(name="smallp", bufs=4) as smallp,
            tc.tile_pool(name="ps_sc", bufs=2, space="PSUM") as ps_sc,
            tc.tile_pool(name="dpool", bufs=1, space="DRAM") as dpool,
        ):
            # Q'^T[d,q], one tile per 512-query half so the scores phase
            # never waits on the other half's psum drain
            qpt = [
                persist.tile([P, DC, 512], FP16, name=f"qpt{s}")
                for s in range(2)
            ]
            # V' in rank-relative key order: vpA = own half (local only),
            # vpB = peer half (from the gather)
            vpA = persist.tile([P, HKC, OUT + 2], FP16, name="vpA")
            vpB = persist.tile([P, HKC, OUT + 2], FP16, name="vpB")

            vpb_in = dpool.tile([P, HKC, OUT + 2], FP16)
            vpb_out = dpool.tile([2, P, HKC, OUT + 2], FP16)

            wv_sb = wpool.tile([P, 2, DC, 384], FP16)
            mw_sb = wpool.tile([P, DC, D], FP16)

            # HAM warmup; memset on vector (earliest-idle engine) so the
            # ramp matmuls can start right after the preamble barrier.
            warm = wpool.tile([P, 512], FP16, name="warm")
            nc.vector.memset(warm, 1.0)

            # Pre-warm the scalar DMA queue with a tiny transfer: its
            # only other use is the final output chunk, which otherwise
            # pays the ~1.6us first-doorbell queue-start latency right
            # on the critical tail.
            qwarm = wpool.tile([P, 8], FP16, name="qwarm")
            nc.scalar.dma_start(out=qwarm, in_=mw[:, 0:8])

            ones_sc = persist.tile([P, 1], F32, name="ones_sc")
            nc.vector.memset(ones_sc, 1.0)
            zero_sc = persist.tile([P, 1], F32, name="zero_sc")
            nc.vector.memset(zero_sc, 0.0)

            ets = {}
            with tc.tile_pool(name="psa", bufs=5, space="PSUM") as psa:
                wps = psa.tile([P, 512], F32, name="wps", bufs=1)
                for i in range(9):
                    nc.tensor.matmul(
                        wps, warm[:, 0:P], warm, start=(i == 0), stop=(i == 8)
                    )

                # All input DMAs ride ONE queue (sync) in strict NEED
                # order: the 16 DMA engines are shared across queues, so
                # a second concurrent queue would steal ~half the
                # bandwidth from the critical stream (measured: A1
                # starved when mw/slab2 ran on the scalar queue early).
                # Chunking wv0/slab0 lets the first ps1 matmul wait on
                # only ~540KB instead of ~1.4MB.
                kslab_tiles = [
                    slabs.tile([P, 4, DC, P], FP16, tag="slab", name=f"kslab{s}")
                    for s in range(4)
                ]
                nc.sync.dma_start(
                    out=wv_sb[:, 0, 0:4], in_=wvi[:, 0 : 4 * 384]
                )
                nc.sync.dma_start(
                    out=kslab_tiles[0][:, 0:2], in_=xh[0][:, 0 : 2 * DC * P]
                )
                nc.sync.dma_start(
                    out=wv_sb[:, 0, 4:6], in_=wvi[:, 4 * 384 : DC * 384]
                )
                nc.sync.dma_start(
                    out=kslab_tiles[0][:, 2:4], in_=xh[0][:, 2 * DC * P :]
                )
                nc.sync.dma_start(
                    out=kslab_tiles[1][:, 0:2], in_=xh[1][:, 0 : 2 * DC * P]
                )
                nc.sync.dma_start(
                    out=kslab_tiles[1][:, 2:4], in_=xh[1][:, 2 * DC * P :]
                )
                nc.sync.dma_start(
                    out=wv_sb[:, 1], in_=wvi[:, DC * 384 :]
                )
                nc.sync.dma_start(out=mw_sb, in_=mw[:, :])
                nc.sync.dma_start(out=kslab_tiles[3], in_=xh[3])
                nc.sync.dma_start(out=kslab_tiles[2], in_=xh[2])

                # ---- A1: V' own half (earliest -> feeds the gather) ----
                # kc0/kc1 run their wv0a-covered dc 0:4 halves FIRST
                # (two open psum groups on different banks): kc1's dc0-3
                # fills the ~1us wait for the wv0b chunk (dc 4-5) that a
                # straight kc0 dc-loop exposes right at kernel start.
                ps_first = [
                    psa.tile([P, 512], F32, tag="psa", name=f"ps1f{k}")
                    for k in range(2)
                ]
                for k in range(2):
                    for dc in range(4):
                        nc.tensor.matmul(
                            ps_first[k][:, 0:384],
                            kslab_tiles[0][:, k, dc, :],
                            wv_sb[:, 0, dc, :],
                            start=(dc == 0),
                            stop=False,
                        )
                for k in range(2):
                    for dc in range(4, DC):
                        nc.tensor.matmul(
                            ps_first[k][:, 0:384],
                            kslab_tiles[0][:, k, dc, :],
                            wv_sb[:, 0, dc, :],
                            start=False,
                            stop=(dc == DC - 1),
                        )
                    nc.vector.tensor_copy(
                        vpA[:, k, 0:384], ps_first[k][:, 0:384]
                    )
                for kc in range(2, HKC):
                    slab = kslab_tiles[kc // 4]
                    ps1 = psa.tile([P, 512], F32, tag="psa")
                    for dc in range(DC):
                        nc.tensor.matmul(
                            ps1[:, 0:384],
                            slab[:, kc % 4, dc, :],
                            wv_sb[:, 0, dc, :],
                            start=(dc == 0),
                            stop=(dc == DC - 1),
                        )
                    nc.vector.tensor_copy(vpA[:, kc, 0:384], ps1[:, 0:384])
                for kc in range(HKC):
                    slab = kslab_tiles[kc // 4]
                    ps2 = psa.tile([P, 512], F32, tag="psa")
                    for dc in range(DC):
                        nc.tensor.matmul(
                            ps2[:, 0:384],
                            slab[:, kc % 4, dc, :],
                            wv_sb[:, 1, dc, :],
                            start=(dc == 0),
                            stop=(dc == DC - 1),
                        )
                    nc.vector.tensor_copy(vpA[:, kc, 384:OUT], ps2[:, 0:384])
                    nc.vector.tensor_copy(vpA[:, kc, OUT : OUT + 1], ones_sc)
                    nc.vector.tensor_copy(
                        vpA[:, kc, OUT + 1 : OUT + 2], zero_sc
                    )
                    nc.gpsimd.dma_start(
                        out=vpb_in[:, kc, :], in_=vpA[:, kc, :]
                    )
                nc.gpsimd.collective_compute(
                    "AllGather",
                    mybir.AluOpType.bypass,
                    replica_groups=PAIRS,
                    ins=[vpb_in.opt()],
                    outs=[vpb_out.opt()],
                )
                # Peer-half readback: the gather output is rank-ordered,
                # so rank r's peer sits in slot 1-r. Two complementary
                # predicated DMAs keep the APs static; the skipped DMA
                # still increments the semaphore, so downstream waits
                # count identically on both ranks. Both on the sync
                # queue (single-queue writers keep the matmul wait).
                me = nc.sync.partition_id() % 2
                nc.sync.dma_start(out=vpB[:], in_=vpb_out[0], cond=me)
                nc.sync.dma_start(
                    out=vpB[:], in_=vpb_out[1], cond=(me + 1) % 2
                )

                # ---- A2: Q'^T = (x_q M)^T own half ----
                for s in range(2):
                    slab = kslab_tiles[s]
                    for oc in range(DC):
                        ps = psa.tile([P, 512], F32, tag="psa")
                        for dc in range(DC):
                            nc.tensor.matmul(
                                ps,
                                mw_sb[:, dc, oc * P : (oc + 1) * P],
                                slab[:, :, dc, :],
                                start=(dc == 0),
                                stop=(dc == DC - 1),
                            )
                        nc.vector.tensor_copy(qpt[s][:, oc, :], ps)

                # ---- scoresT: contracts over d, stationary = xh slab
                # chunks (rank-relative key order), moving = Q'^T. Runs
                # inside the psa scope (ps_sc has its own banks) so no
                # pool-close fence sits between A2 and scores.
                for bi in range(2):
                    for kc in range(KC):
                        kslab = kslab_tiles[kc // 4]
                        st = ps_sc.tile([P, 512], F32, tag="sc")
                        for dc in range(DC):
                            nc.tensor.matmul(
                                st,
                                kslab[:, kc % 4, dc, :],
                                qpt[bi][:, dc, :],
                                start=(dc == 0),
                                stop=(dc == DC - 1),
                            )
                        et = expp.tile(
                            [P, 512], FP16, tag="exp", name=f"et{bi}_{kc}"
                        )
                        nc.scalar.activation(
                            et,
                            st,
                            mybir.ActivationFunctionType.Exp,
                            scale=0.125 / M_SCALE,
                        )
                        ets[(bi, kc)] = et

            # ---- out phase: psa's banks freed above feed ps_out; the
            # pool-close fence overlaps the V-gather wait.
            with (
                tc.tile_pool(name="ps_den", bufs=3, space="PSUM") as ps_den,
                tc.tile_pool(name="ps_av", bufs=3, space="PSUM") as ps_av,
            ):
                # 8 q-chunks of 128. Denominator run (cols 512:770) goes
                # FIRST into its OWN 1-bank psum tile so the recip and
                # 512:768 normalize never WAR-block the 0:512 run (a
                # shared tile serialized run2 behind the normalize —
                # measured ~0.7us per exposed boundary); kc 0-7 read vpA
                # (local), kc 8-15 read vpB (gathered peer half).
                def vsrc(kc):
                    return vpA if kc < HKC else vpB

                for j in range(NQ // P):
                    bi, jj = j // 4, j % 4
                    opd = ps_den.tile([P, 258], F32, tag="den", name=f"den{j}")
                    for kc in range(KC):
                        nc.tensor.matmul(
                            opd,
                            ets[(bi, kc)][:, jj * P : (jj + 1) * P],
                            vsrc(kc)[:, kc % HKC, 512 : OUT + 2],
                            start=(kc == 0),
                            stop=(kc == KC - 1),
                        )
                    recip = smallp.tile([P, 1], F32, tag="recip")
                    nc.vector.reciprocal(recip, opd[:, 256:257])
                    ob = obp.tile([P, OUT], FP16, tag="ob")
                    nc.vector.tensor_scalar_mul(
                        ob[:, 512:OUT], opd[:, 0:256], recip
                    )
                    if j == NQ // P - 1:
                        # last chunk: ship cols 512:768 now (scalar
                        # queue), run the 0:512 accumulation as two
                        # 256-col runs in separate psum tiles so the
                        # first half's normalize + DMA overlap the
                        # second half's matmuls.
                        nc.scalar.dma_start(
                            out=out[j * P : (j + 1) * P, 512:OUT],
                            in_=ob[:, 512:OUT],
                        )
                        for lo, hi, eng in (
                            (0, 256, nc.sync),
                            (256, 448, nc.scalar),
                            (448, 512, nc.sync),
                        ):
                            opa = ps_av.tile(
                                [P, hi - lo], F32, tag="av", name=f"av{j}_{lo}"
                            )
                            for kc in range(KC):
                                nc.tensor.matmul(
                                    opa,
                                    ets[(bi, kc)][:, jj * P : (jj + 1) * P],
                                    vsrc(kc)[:, kc % HKC, lo:hi],
                                    start=(kc == 0),
                                    stop=(kc == KC - 1),
                                )
                            nc.vector.tensor_scalar_mul(
                                ob[:, lo:hi], opa, recip
                            )
                            eng.dma_start(
                                out=out[j * P : (j + 1) * P, lo:hi],
                                in_=ob[:, lo:hi],
                            )
                    else:
                        opa = ps_av.tile([P, 512], F32, tag="av", name=f"av{j}")
                        for kc in range(KC):
                            nc.tensor.matmul(
                                opa,
                                ets[(bi, kc)][:, jj * P : (jj + 1) * P],
                                vsrc(kc)[:, kc % HKC, 0:512],
                                start=(kc == 0),
                                stop=(kc == KC - 1),
                            )
                        nc.vector.tensor_scalar_mul(
                            ob[:, 0:512], opa, recip
                        )
                        nc.sync.dma_start(
                            out=out[j * P : (j + 1) * P, :], in_=ob
                        )
    nc.finalize()
    return nc


_NC_CACHE = None


def _get_nc():
    global _NC_CACHE
    if _NC_CACHE is None:
        _NC_CACHE = build_attention_nc()
    return _NC_CACHE


def _xh_layout(a2d):
    """[D, 2048] -> [4, P, 4*DC*128], quarter-major slabs: the kc-th
    128-token quarter of a slab is a contiguous DMA prefix."""
    t = a2d.reshape(DC, P, 4, 4, P)  # dc p s q t
    t = t.transpose(2, 1, 3, 0, 4)  # s p q dc t
    return np.ascontiguousarray(t.reshape(4, P, 4 * DC * P))


def _wv_layout(a2d):
    """[D, 768] -> [P, 2*DC*384], column-half-major."""
    t = a2d.reshape(DC, P, 2, 384)  # dc p h c
    t = t.transpose(1, 2, 0, 3)  # p h dc c
    return np.ascontiguousarray(t.reshape(P, 2 * DC * 384))


def _mw_layout(a2d):
    """[D, D] -> [P, DC*D], dc-major."""
    t = a2d.reshape(DC, P, D).transpose(1, 0, 2)
    return np.ascontiguousarray(t.reshape(P, DC * D))


def make_in_maps(x, kernel):
    x = np.asarray(x, dtype=np.float32)
    w = np.asarray(kernel, dtype=np.float32)
    mw16 = (M_SCALE * (w[0] @ w[1].T)).astype(np.float16)
    mw = _mw_layout(mw16)
    wv = _wv_layout(w[2].astype(np.float16))
    in_maps = []
    for core in range(N_CORES):
        b, half = core // 2, core % 2
        xt16 = x[b].T.astype(np.float16)
        # rank-relative key order: own 1024 tokens first, then peer's
        own = xt16[:, half * NQ : (half + 1) * NQ]
        peer = xt16[:, (1 - half) * NQ : (2 - half) * NQ]
        xh = _xh_layout(np.concatenate([own, peer], axis=1))
        in_maps.append({"xh": xh, "mw": mw, "wvi": wv})
    return in_maps


def assemble_output(results):
    out = np.empty((B, N, OUT), dtype=np.float32)
    for core in range(N_CORES):
        b, half = core // 2, core % 2
        out[b, half * NQ : (half + 1) * NQ, :] = results[core]["out"]
    return out


def run_on_hw(x, kernel, trace=False):
    nc = _get_nc()
    res = run_bass_kernel_spmd(
        nc, make_in_maps(x, kernel), list(range(N_CORES)), trace=trace
    )
    return assemble_output(res.results), res


def kernel(x, kernel):
    out, _ = run_on_hw(x, kernel, trace=False)
    return out

